# revision 1
# baseline (speedup 1.0000x reference)
"""Trainium2 Bass kernel for nn_Block_47193100648803.

Contract: kernel(**inputs) takes FULL unsharded inputs (numpy), returns the
FULL (N, O, T, V) output. Internally shards data-parallel over N across the
8 NeuronCores (one batch element per core, weights replicated).

Layout: channels on SBUF partitions (C=256 -> 2 half-tiles of 128), tokens on
the free axis. The temporal-window unfold is expressed with overlapping
strided access patterns (no data movement). LayerNorm stats are computed with
PE ones-matmuls (sum + partition-broadcast fused); the FFN/temporal LNs are
folded into the following matmul's weights (W1g = g*W1 plus an outer-product
mean correction accumulated in PSUM). Softmaxes run in group-on-partition
layout reached via DRAM-bounce transpose DMAs. Wt@Wp is pre-fused. All
matmuls run fp32r (full PE rate); bf16 only for non-matmul operand storage.
"""

import os
import sys

import numpy as np

for _p in ("/opt/trn_rl_repo", "/root/.axon_site/_ro/trn_rl_repo"):
    if os.path.isdir(_p) and _p not in sys.path:
        sys.path.append(_p)

import concourse.bass as bass
import concourse.tile as tile
from concourse import bacc, bass_utils, mybir
from concourse.masks import make_identity

f32 = mybir.dt.float32
f32r = mybir.dt.float32r
bf16 = mybir.dt.bfloat16
AF = mybir.ActivationFunctionType
ALU = mybir.AluOpType
AX = mybir.AxisListType

# ---- problem constants (hardcoded per spec) ----
N_CORES = 8
C, T, V = 256, 128, 25
H = 8
W = 3
O = 256
L = W * V                 # 75
FT = T + 2                # 130 padded frames
F = FT * V                # 3250 real frame columns (zero pads at both ends)
F_PAD = 3328              # allocated frame columns (8 * 416, fp32r-even subs)
G = T                     # 128 groups per core
GL = G * L                # 9600 group-stage columns
SCALE = 1.0 / (32.0 ** 0.5)
EPS = 1e-5

FSUB = 416                # phase-1 matmul column tile (even, 256..512)
N_FSUB = F_PAD // FSUB    # 8
CH_G = 16                 # groups per chunk in phase 2
N_CH = G // CH_G          # 8
CH = CH_G * L             # 1200
SUB_G = 4                 # groups per matmul sub-tile
SUB = SUB_G * L           # 300
N_SUB = CH_G // SUB_G     # 4
SUBW = 400                # wide matmul sub (fp32r-even, 256..512)
N_SUBW = CH // SUBW       # 3
# bank-aligned dst slices for chunk-wide [128, CH] psum accumulation
BANK_SUBS = [(0, 512), (512, 512), (1024, 176)]


def _r(ap):
    return ap.bitcast(f32r)


def _view(t, offset, dims):
    """AP view on tile t: partition dim kept, free dims replaced."""
    return bass.AP(tensor=t.tensor, offset=t.offset + offset, ap=[t.ap[0]] + dims)


def unf(t, g0, gc):
    """Overlapping window view [128, gc, W, V] on a [128, F] frame tile."""
    return _view(t, g0 * V, [[V, gc], [V, W], [1, V]])


def seg(t, g0, gc):
    """[128, gc, L] view on a [128, GL] or chunk tile starting at group g0
    (g0 relative to tile origin)."""
    return _view(t, g0 * L, [[L, gc], [1, L]])


def bc_g(t, g0, gc):
    """Broadcast per-(c,g) [128, G] tile over L -> [128, gc, L] (step-0)."""
    return _view(t, g0, [[1, gc], [0, L]])


def build(nc):
    x_d = nc.dram_tensor("x", [C, T, V], f32, kind="ExternalInput").ap()
    wd = {}
    for nm in ["Wq", "Wk", "Wv", "Wt", "Wp", "W1", "W2", "c1_w"]:
        wd[nm] = nc.dram_tensor(nm, [C, C], f32, kind="ExternalInput").ap()
    wd["Wqa"] = nc.dram_tensor("Wqa", [C, H], f32, kind="ExternalInput").ap()
    wd["Wka"] = nc.dram_tensor("Wka", [C, H], f32, kind="ExternalInput").ap()
    wd["c2_w"] = nc.dram_tensor("c2_w", [W, C, O], f32, kind="ExternalInput").ap()
    bnames = ["ln1_g", "ln1_b", "bq", "bk", "bv", "bt", "bp", "ffn_g", "ffn_b",
              "b1", "b2", "tn_g", "tn_b", "c1_b", "c2_b"]
    for nm in bnames:
        wd[nm] = nc.dram_tensor(nm, [C], f32, kind="ExternalInput").ap()
    wd["bqa"] = nc.dram_tensor("bqa", [H], f32, kind="ExternalInput").ap()
    wd["bka"] = nc.dram_tensor("bka", [H], f32, kind="ExternalInput").ap()
    out_d = nc.dram_tensor("out", [O, T, V], f32, kind="ExternalOutput").ap()

    qa_d = nc.dram_tensor("qa_scr", [H, F_PAD], f32).ap()
    qw_d = nc.dram_tensor("qw_scr", [H, GL], bf16).ap()
    ka_d = nc.dram_tensor("ka_scr", [H, GL], f32).ap()
    kw_d = nc.dram_tensor("kw_scr", [H, GL], bf16).ap()
    row_d = nc.dram_tensor("row_scr", [3, C], f32).ap()

    with tile.TileContext(nc) as tc:
        with (
            tc.tile_pool(name="consts", bufs=1) as cp,
            tc.tile_pool(name="data", bufs=1) as dp,
        ):
            # ---------- input load first (weights go on other DMA queues) ----
            p1x_cm = tc.tile_pool(name="p1_x", bufs=1)
            p1x = p1x_cm.__enter__()
            x_f = [p1x.tile([128, F_PAD], f32, tag=f"x_f{hh}", name=f"x_f{hh}")
                   for hh in range(2)]
            qa_f = p1x.tile([H, F_PAD], f32, tag="qa_f", name="qa_f")
            for hh in range(2):
                nc.gpsimd.dma_start(out=_r(x_f[hh][:, V:F - V]),
                                    in_=_r(x_d[hh * 128:(hh + 1) * 128, :, :]))

            # ---------- weights / constants ----------
            wt = {}
            for nm in ["Wq", "Wk", "Wv", "Wp", "W1", "W2", "c1_w"]:
                wt[nm] = [cp.tile([128, C], f32, tag=f"w_{nm}{kh}", name=f"w_{nm}{kh}")
                          for kh in range(2)]
                for kh in range(2):
                    nc.scalar.dma_start(out=_r(wt[nm][kh]),
                                        in_=_r(wd[nm][kh * 128:(kh + 1) * 128, :]))
            for nm in ["Wqa", "Wka"]:
                wt[nm] = [cp.tile([128, H], f32, tag=f"w_{nm}{kh}", name=f"w_{nm}{kh}")
                          for kh in range(2)]
                for kh in range(2):
                    nc.scalar.dma_start(out=_r(wt[nm][kh]),
                                        in_=_r(wd[nm][kh * 128:(kh + 1) * 128, :]))
            c2t = []
            for w in range(W):
                c2t.append([cp.tile([128, O], f32, tag=f"w_c2_{w}{kh}", name=f"w_c2_{w}{kh}")
                            for kh in range(2)])
                for kh in range(2):
                    nc.scalar.dma_start(out=_r(c2t[w][kh]),
                                        in_=_r(wd["c2_w"][w, kh * 128:(kh + 1) * 128, :]))

            def load_bias_col(nm):
                t = cp.tile([128, 2], f32, tag=f"b_{nm}", name=f"b_{nm}")
                src = bass.AP(tensor=wd[nm].tensor, offset=wd[nm].offset,
                              ap=[[1, 128], [128, 2]])
                nc.scalar.dma_start(out=t, in_=src)
                return t

            bias = {nm: load_bias_col(nm) for nm in bnames}
            for nm in ["bqa", "bka"]:
                t = cp.tile([H, 1], f32, tag=f"b_{nm}", name=f"b_{nm}")
                nc.sync.dma_start(out=t, in_=wd[nm])
                bias[nm] = t

            eps_t = cp.tile([128, 1], f32, tag="eps", name="eps_t")
            nc.vector.memset(eps_t, EPS)

            def fill_r(t, value):
                # constant fill with an f32r-typed output (plain Memset cannot
                # emit f32r): Copy(in*0 + value) ignores the uninitialized in_
                nc.scalar.activation(out=_r(t), in_=_r(t), func=AF.Copy,
                                     bias=float(value), scale=0.0)

            onesC = cp.tile([128, 128], f32, tag="onesC", name="onesC")
            fill_r(onesC, 1.0 / C)
            onesC_b = cp.tile([128, 128], bf16, tag="onesC_b", name="onesC_b")
            nc.scalar.activation(out=onesC_b, in_=onesC, func=AF.Copy)
            w2b = [cp.tile([128, C], bf16, tag=f"w2b{kh}", name=f"w2b{kh}") for kh in range(2)]
            wkab = [cp.tile([128, H], bf16, tag=f"wkab{kh}", name=f"wkab{kh}") for kh in range(2)]
            negones = cp.tile([128, 1], f32, tag="negones", name="negones")
            fill_r(negones, -1.0)
            negones_b = cp.tile([128, 1], bf16, tag="negones_b", name="negones_b")
            nc.scalar.activation(out=negones_b, in_=negones, func=AF.Copy)

            # folded weights: W1g = ffn_g*W1, c1g = tn_g*c1_w
            w1g = [cp.tile([128, C], bf16, tag=f"w1g{kh}", name=f"w1g{kh}") for kh in range(2)]
            c1g = [cp.tile([128, C], bf16, tag=f"c1g{kh}", name=f"c1g{kh}") for kh in range(2)]
            for kh in range(2):
                nc.vector.tensor_scalar_mul(w1g[kh], wt["W1"][kh], bias["ffn_g"][:, kh:kh + 1])
                nc.vector.tensor_scalar_mul(c1g[kh], wt["c1_w"][kh], bias["tn_g"][:, kh:kh + 1])
                nc.scalar.activation(out=w2b[kh], in_=wt["W2"][kh], func=AF.Copy)
                nc.scalar.activation(out=wkab[kh], in_=wt["Wka"][kh], func=AF.Copy)

            wtp = [cp.tile([128, C], bf16, tag=f"wtp{kh}", name=f"wtp{kh}") for kh in range(2)]
            negg = [cp.tile([1, C], bf16, tag=f"negg{i}", name=f"negg{i}")
                    for i in range(2)]  # [-G1], [-Gc1]

            # ---------- setup-scoped: Wtp = Wt@Wp, bias rows ----------
            with (
                tc.tile_pool(name="setup_sb", bufs=1) as sp,
                tc.tile_pool(name="setup_ps", bufs=2, space="PSUM") as spp,
            ):
                wtw = [sp.tile([128, C], f32, tag=f"wt{kh}", name=f"wtw{kh}")
                       for kh in range(2)]
                for kh in range(2):
                    nc.sync.dma_start(out=wtw[kh],
                                      in_=wd["Wt"][kh * 128:(kh + 1) * 128, :])
                ident = sp.tile([128, 128], f32, tag="ident", name="ident")
                make_identity(nc, ident)

                for kh in range(2):
                    pacc = spp.tile([128, C], f32, tag="wtp_acc", name="pacc")
                    for mh in range(2):
                        ptr = spp.tile([128, 128], f32, tag="tr", name="ptr")
                        nc.tensor.transpose(ptr, wtw[kh][:, mh * 128:(mh + 1) * 128], ident)
                        a_t = sp.tile([128, 128], f32, tag="a_t", name="a_t")
                        nc.scalar.activation(out=_r(a_t), in_=ptr, func=AF.Copy)
                        nc.tensor.matmul(pacc, _r(a_t), _r(wt["Wp"][mh]),
                                         start=(mh == 0), stop=(mh == 1))
                    nc.scalar.activation(out=wtp[kh], in_=pacc, func=AF.Copy)

                def colvec(nm, kh):
                    t = sp.tile([128, 1], f32, tag=f"cv_{nm}{kh}", name=f"cv_{nm}{kh}")
                    src = bass.AP(tensor=wd[nm].tensor, offset=wd[nm].offset + kh * 128,
                                  ap=[[1, 128], [128, 1]])
                    nc.sync.dma_start(out=_r(t), in_=_r(src))
                    return t

                def rowvec(nm):
                    t = sp.tile([1, C], f32, tag=f"rv_{nm}", name=f"rv_{nm}")
                    nc.sync.dma_start(out=t, in_=wd[nm])
                    return t

                for i, (bnm, wmat, addnm) in enumerate([
                    ("bt", wt["Wp"], "bp"),
                    ("ffn_b", wt["W1"], "b1"),
                    ("tn_b", wt["c1_w"], "c1_b"),
                ]):
                    pr = spp.tile([1, C], f32, tag="rowacc", name="pr")
                    for kh in range(2):
                        nc.tensor.matmul(pr, _r(colvec(bnm, kh)), _r(wmat[kh]),
                                         start=(kh == 0), stop=(kh == 1))
                    row_i = sp.tile([1, C], f32, tag=f"row_i{i}", name=f"row_i{i}")
                    nc.vector.tensor_add(row_i, pr, rowvec(addnm))
                    nc.sync.dma_start(out=row_d[i:i + 1, :], in_=row_i)

                for i, wmat in enumerate([w1g, c1g]):
                    pg = spp.tile([1, C], f32, tag="rowacc", name="pg")
                    for kh in range(2):
                        nc.tensor.matmul(pg, negones_b, wmat[kh],
                                         start=(kh == 0), stop=(kh == 1))
                    nc.scalar.activation(out=negg[i], in_=pg, func=AF.Copy)

            # bounce bias rows back into per-partition [128, 2] layout
            btp_t = cp.tile([128, 2], f32, tag="btp", name="btp_t")
            B1_t = cp.tile([128, 2], f32, tag="B1", name="B1_t")
            Bc1_t = cp.tile([128, 2], f32, tag="Bc1", name="Bc1_t")
            for i, t in enumerate([btp_t, B1_t, Bc1_t]):
                src = bass.AP(tensor=row_d.tensor, offset=row_d.offset + i * C,
                              ap=[[1, 128], [128, 1]])
                nc.sync.dma_start(out=t[:, 0:1], in_=src)
                src2 = bass.AP(tensor=row_d.tensor, offset=row_d.offset + i * C + 128,
                               ap=[[1, 128], [128, 1]])
                nc.sync.dma_start(out=t[:, 1:2], in_=src2)

            # ---------- persistent activations ----------
            q_f = [dp.tile([128, F_PAD], bf16, tag=f"q_f{hh}", name=f"q_f{hh}") for hh in range(2)]
            k_f = [dp.tile([128, F_PAD], bf16, tag=f"k_f{hh}", name=f"k_f{hh}") for hh in range(2)]
            v_f = [dp.tile([128, F_PAD], bf16, tag=f"v_f{hh}", name=f"v_f{hh}") for hh in range(2)]
            px_f = [dp.tile([128, F_PAD], bf16, tag=f"px_f{hh}", name=f"px_f{hh}") for hh in range(2)]
            pq_t = [dp.tile([128, G], f32, tag=f"pq{hh}", name=f"pq{hh}") for hh in range(2)]
            pk_t = [dp.tile([128, G], f32, tag=f"pk{hh}", name=f"pk{hh}") for hh in range(2)]
            pq_b = [dp.tile([128, G], bf16, tag=f"pqb{hh}", name=f"pqb{hh}") for hh in range(2)]
            pk_b = [dp.tile([128, G], bf16, tag=f"pkb{hh}", name=f"pkb{hh}") for hh in range(2)]

            # ---------- phase 1: per-frame pipeline ----------
            with (
                tc.tile_pool(name="p1_sb", bufs=2) as p1,
                tc.tile_pool(name="p1_ps", bufs=1, space="PSUM") as pp1,
                tc.tile_pool(name="p1_mm", bufs=4, space="PSUM") as pp1m,
            ):
                for hh in range(2):
                    fill_r(x_f[hh][:, 0:V], 0.0)
                    fill_r(x_f[hh][:, F - V:F_PAD], 0.0)

                for s in range(N_FSUB):
                    sl = slice(s * FSUB, (s + 1) * FSUB)
                    x2 = [p1.tile([128, FSUB], f32, tag=f"x2_{hh}", name=f"x2_{hh}")
                          for hh in range(2)]
                    for hh in range(2):
                        nc.vector.scalar_tensor_tensor(
                            out=_r(x2[hh]), in0=x_f[hh][:, sl], scalar=1.0,
                            in1=x_f[hh][:, sl], op0=ALU.mult, op1=ALU.mult)
                    pmean = pp1.tile([128, FSUB], f32, tag="pmean", name="pmean")
                    pmsq = pp1.tile([128, FSUB], f32, tag="pmsq", name="pmsq")
                    for hh in range(2):
                        nc.tensor.matmul(pmean, _r(onesC), _r(x_f[hh][:, sl]),
                                         start=(hh == 0), stop=(hh == 1))
                    for hh in range(2):
                        nc.tensor.matmul(pmsq, _r(onesC), _r(x2[hh]),
                                         start=(hh == 0), stop=(hh == 1))
                    m2 = p1.tile([128, FSUB], f32, tag="m2", name="m2")
                    nc.scalar.activation(out=m2, in_=pmean, func=AF.Square)
                    var = p1.tile([128, FSUB], f32, tag="var", name="var")
                    nc.vector.tensor_sub(var, pmsq, m2)
                    lnv = p1.tile([128, FSUB], f32, tag="sd", name="lnv")
                    nc.scalar.activation(out=lnv, in_=var, func=AF.Ln, bias=eps_t)
                    rstd = p1.tile([128, FSUB], f32, tag="rstd", name="rstd")
                    nc.scalar.activation(out=rstd, in_=lnv, func=AF.Exp, scale=-0.5)
                    nx = []
                    for hh in range(2):
                        xc = p1.tile([128, FSUB], f32, tag=f"xc{hh}", name=f"xc{hh}")
                        nc.vector.tensor_sub(xc, x_f[hh][:, sl], pmean)
                        xg = p1.tile([128, FSUB], f32, tag=f"xg{hh}", name=f"xg{hh}")
                        nc.vector.scalar_tensor_tensor(
                            out=xg, in0=xc, scalar=bias["ln1_g"][:, hh:hh + 1],
                            in1=rstd, op0=ALU.mult, op1=ALU.mult)
                        nxh = p1.tile([128, FSUB], f32, tag=f"nx{hh}", name=f"nx{hh}")
                        nc.vector.tensor_scalar_add(_r(nxh), xg, bias["ln1_b"][:, hh:hh + 1])
                        nx.append(nxh)
                    q32 = []
                    for mh in range(2):
                        pq_ = pp1m.tile([128, FSUB], f32, tag="mm", name="pq_")
                        for kh in range(2):
                            nc.tensor.matmul(pq_, _r(wt["Wq"][kh][:, mh * 128:(mh + 1) * 128]),
                                             _r(nx[kh]), start=(kh == 0), stop=(kh == 1))
                        qh = p1.tile([128, FSUB], f32, tag=f"q32_{mh}", name=f"q32_{mh}")
                        nc.scalar.activation(out=_r(qh), in_=pq_, func=AF.Identity,
                                             bias=bias["bq"][:, mh:mh + 1])
                        q32.append(qh)
                        nc.vector.tensor_copy(q_f[mh][:, sl], qh)
                    for nm, bnm, dst in [("Wk", "bk", k_f), ("Wv", "bv", v_f)]:
                        for mh in range(2):
                            pm_ = pp1m.tile([128, FSUB], f32, tag="mm", name="pm_")
                            for kh in range(2):
                                nc.tensor.matmul(pm_,
                                                 _r(wt[nm][kh][:, mh * 128:(mh + 1) * 128]),
                                                 _r(nx[kh]), start=(kh == 0), stop=(kh == 1))
                            nc.scalar.activation(out=dst[mh][:, sl], in_=pm_,
                                                 func=AF.Identity,
                                                 bias=bias[bnm][:, mh:mh + 1])
                    pqa = pp1.tile([H, FSUB], f32, tag="pqa", name="pqa")
                    for kh in range(2):
                        nc.tensor.matmul(pqa, _r(wt["Wqa"][kh]), _r(nx[kh]),
                                         start=(kh == 0), stop=(kh == 1))
                    nc.scalar.activation(out=qa_f[:, sl], in_=pqa, func=AF.Identity,
                                         bias=bias["bqa"])
                    # px = q@Wp + btp + x   (pre-added residual path for attn)
                    for mh in range(2):
                        pp_ = pp1m.tile([128, FSUB], f32, tag="mm", name="pp_")
                        for kh in range(2):
                            nc.tensor.matmul(pp_, _r(wt["Wp"][kh][:, mh * 128:(mh + 1) * 128]),
                                             _r(q32[kh]), start=(kh == 0), stop=(kh == 1))
                        nc.vector.scalar_tensor_tensor(
                            out=px_f[mh][:, sl], in0=pp_, scalar=btp_t[:, mh:mh + 1],
                            in1=x_f[mh][:, sl], op0=ALU.add, op1=ALU.add)
                nc.sync.dma_start(out=qa_d, in_=qa_f)

            p1x_cm.__exit__(None, None, None)

            # ---------- global qw softmax (batched over all groups) ----------
            with tc.tile_pool(name="smq", bufs=1) as smq:
                ag = smq.tile([G, H * L], f32, tag="ag", name="ag_q")
                qa_gather_all = bass.AP(
                    tensor=qa_d.tensor, offset=qa_d.offset,
                    ap=[[V, G], [F_PAD, H], [V, W], [1, V]])
                nc.gpsimd.dma_start(out=ag, in_=qa_gather_all)
                ag3 = _view(ag, 0, [[L, H], [1, L]])
                mx = smq.tile([G, H], f32, tag="mx", name="mx_q")
                nc.vector.reduce_max(mx, ag3, axis=AX.X)
                e = smq.tile([G, H * L], f32, tag="e", name="e_q")
                nc.vector.tensor_sub(_view(e, 0, [[L, H], [1, L]]), ag3,
                                     _view(mx, 0, [[1, H], [0, L]]))
                nc.scalar.activation(out=e, in_=e, func=AF.Exp, scale=SCALE)
                sm = smq.tile([G, H], f32, tag="sm", name="sm_q")
                nc.vector.reduce_sum(sm, _view(e, 0, [[L, H], [1, L]]), axis=AX.X)
                rs = smq.tile([G, H], f32, tag="rs", name="rs_q")
                nc.vector.reciprocal(rs, sm)
                wgn = smq.tile([G, H * L], bf16, tag="wgn", name="wgn_q")
                nc.vector.scalar_tensor_tensor(
                    out=_view(wgn, 0, [[L, H], [1, L]]),
                    in0=_view(e, 0, [[L, H], [1, L]]), scalar=1.0,
                    in1=_view(rs, 0, [[1, H], [0, L]]),
                    op0=ALU.mult, op1=ALU.mult)
                qw_all = bass.AP(tensor=qw_d.tensor, offset=qw_d.offset,
                                 ap=[[L, G], [GL, H], [1, L]])
                nc.gpsimd.dma_start(out=qw_all, in_=wgn)

            # ---------- phase 2: unified per-chunk pipeline ----------
            with (
                tc.tile_pool(name="p2_sb", bufs=1) as p2,
                tc.tile_pool(name="p2_ps", bufs=2, space="PSUM") as pmm,
                tc.tile_pool(name="p2_ps2", bufs=2, space="PSUM") as pst,
            ):
                def softmax_chunk(src_gather_ap, dst_dram, g0, tagp):
                    """Per-chunk softmax in [128 = 16 groups x 8 heads, L]
                    layout; writes normalized weights to dst_dram[h, cols]."""
                    ag = p2.tile([128, L], f32, tag="sm_ag", bufs=6,
                                 name=f"ag_{tagp}")
                    nc.gpsimd.dma_start(out=ag, in_=src_gather_ap)
                    mx = p2.tile([128, 1], f32, tag="sm_mx", bufs=6,
                                 name=f"mx_{tagp}")
                    nc.vector.reduce_max(mx, ag, axis=AX.X)
                    e = p2.tile([128, L], f32, tag="sm_e", bufs=6,
                                name=f"e_{tagp}")
                    nc.vector.tensor_scalar_sub(e, ag, mx[:, 0:1])
                    nc.scalar.activation(out=e, in_=e, func=AF.Exp, scale=SCALE)
                    sm = p2.tile([128, 1], f32, tag="sm_s", bufs=6,
                                 name=f"sm_{tagp}")
                    nc.vector.reduce_sum(sm, e, axis=AX.X)
                    rs = p2.tile([128, 1], f32, tag="sm_rs", bufs=6,
                                 name=f"rs_{tagp}")
                    nc.vector.reciprocal(rs, sm)
                    wgn = p2.tile([128, L], bf16, tag="sm_w", bufs=6,
                                  name=f"wgn_{tagp}")
                    nc.vector.tensor_scalar_mul(wgn, e, rs[:, 0:1])
                    dst = bass.AP(tensor=dst_dram.tensor,
                                  offset=dst_dram.offset + g0 * L,
                                  ap=[[L, CH_G], [GL, H], [1, L]])
                    nc.gpsimd.dma_start(out=dst, in_=wgn)

                def head_bcast(src_dram, g0, hh, tagp):
                    """[128, CH] tile with partition c reading
                    src_dram[c // 32 (+4*hh), chunk cols] via broadcast DMA."""
                    t = p2.tile([128, CH], bf16, tag="bc", bufs=6,
                                name=f"bc_{tagp}")
                    src = bass.AP(
                        tensor=src_dram.tensor,
                        offset=src_dram.offset + (hh * 4) * GL + g0 * L,
                        ap=[[GL, 4], [0, 32], [1, CH]])
                    nc.sync.dma_start(out=t, in_=src)
                    return t

                def chunk_front(cc):
                    g0 = cc * CH_G
                    col0 = g0 * L

                    # pooled query pq, then kp = k * pq, ka = kp @ Wka
                    kp = []
                    for hh in range(2):
                        qb = head_bcast(qw_d, g0, hh, f"q{hh}")
                        prod = p2.tile([128, CH], bf16, tag="prod", bufs=4,
                                       name="prod")
                        nc.vector.scalar_tensor_tensor(
                            out=_view(prod, 0, [[L, CH_G], [1, L]]),
                            in0=unf(q_f[hh], g0, CH_G), scalar=1.0,
                            in1=_view(qb, 0, [[L, CH_G], [1, L]]),
                            op0=ALU.mult, op1=ALU.mult)
                        nc.vector.reduce_sum(pq_t[hh][:, g0:g0 + CH_G],
                                             _view(prod, 0, [[L, CH_G], [1, L]]),
                                             axis=AX.X)
                        nc.vector.tensor_copy(pq_b[hh][:, g0:g0 + CH_G],
                                              pq_t[hh][:, g0:g0 + CH_G])
                        kph = p2.tile([128, CH], bf16, tag="rhs", bufs=6, name="kph")
                        nc.vector.scalar_tensor_tensor(
                            out=_view(kph, 0, [[L, CH_G], [1, L]]),
                            in0=unf(k_f[hh], g0, CH_G), scalar=1.0,
                            in1=bc_g(pq_b[hh], g0, CH_G),
                            op0=ALU.mult, op1=ALU.mult)
                        kp.append(kph)
                    ka_c = p2.tile([H, CH], f32, tag="ka_c", bufs=2, name="ka_c")
                    for su in range(N_SUBW):
                        pka = pst.tile([H, SUBW], f32, tag="stat", name="pka")
                        for kh in range(2):
                            nc.tensor.matmul(pka, wkab[kh],
                                             kp[kh][:, su * SUBW:(su + 1) * SUBW],
                                             start=(kh == 0), stop=(kh == 1))
                        nc.scalar.activation(out=ka_c[:, su * SUBW:(su + 1) * SUBW],
                                             in_=pka, func=AF.Identity, bias=bias["bka"])
                    nc.gpsimd.dma_start(out=ka_d[:, col0:col0 + CH], in_=ka_c)

                    # kw softmax for this chunk
                    ka_gather = bass.AP(
                        tensor=ka_d.tensor, offset=ka_d.offset + col0,
                        ap=[[L, CH_G], [GL, H], [1, L]])
                    softmax_chunk(ka_gather, kw_d, g0, "k")

                    # pooled key pk, z = v * pk
                    z = []
                    for hh in range(2):
                        kb = head_bcast(kw_d, g0, hh, f"k{hh}")
                        prod = p2.tile([128, CH], bf16, tag="prod", bufs=4,
                                       name="prod2")
                        nc.vector.scalar_tensor_tensor(
                            out=_view(prod, 0, [[L, CH_G], [1, L]]),
                            in0=unf(k_f[hh], g0, CH_G), scalar=1.0,
                            in1=_view(kb, 0, [[L, CH_G], [1, L]]),
                            op0=ALU.mult, op1=ALU.mult)
                        nc.vector.reduce_sum(pk_t[hh][:, g0:g0 + CH_G],
                                             _view(prod, 0, [[L, CH_G], [1, L]]),
                                             axis=AX.X)
                        nc.vector.tensor_copy(pk_b[hh][:, g0:g0 + CH_G],
                                              pk_t[hh][:, g0:g0 + CH_G])
                        zh = p2.tile([128, CH], bf16, tag="ztag", bufs=6, name="zh")
                        nc.vector.scalar_tensor_tensor(
                            out=_view(zh, 0, [[L, CH_G], [1, L]]),
                            in0=unf(v_f[hh], g0, CH_G), scalar=1.0,
                            in1=bc_g(pk_b[hh], g0, CH_G),
                            op0=ALU.mult, op1=ALU.mult)
                        z.append(zh)
                    return z

                def chunk_back(cc, z):
                    g0 = cc * CH_G
                    col0 = g0 * L

                    def layer(rhs_pair, wpair, outer_row=None):
                        """Chunk-wide psum per out-half of rhs @ W (+ optional
                        K=1 outer-product accumulation); bank-aligned dst
                        slices, kh-outer for stationary reuse."""
                        ps = []
                        for mh in range(2):
                            pm = pmm.tile([128, CH], f32, tag="mm", bufs=2, name="pm")
                            last = outer_row is None
                            for kh in range(2):
                                for o0, w_ in BANK_SUBS:
                                    cs = slice(o0, o0 + w_)
                                    nc.tensor.matmul(
                                        pm[:, cs],
                                        wpair[kh][:, mh * 128:(mh + 1) * 128],
                                        rhs_pair[kh][:, cs],
                                        start=(kh == 0), stop=(kh == 1) and last)
                            if outer_row is not None:
                                row, vec = outer_row
                                for o0, w_ in BANK_SUBS:
                                    cs = slice(o0, o0 + w_)
                                    nc.tensor.matmul(
                                        pm[:, cs],
                                        row[0:1, mh * 128:(mh + 1) * 128],
                                        vec[0:1, cs],
                                        start=False, stop=True)
                            ps.append(pm)
                        return ps

                    # att = z @ Wtp + px_unf
                    patt = layer(z, wtp)
                    att = []
                    for mh in range(2):
                        ah = p2.tile([128, CH], bf16, tag="att", bufs=2, name="att")
                        nc.vector.scalar_tensor_tensor(
                            out=_view(ah, 0, [[L, CH_G], [1, L]]),
                            in0=_view(patt[mh], 0, [[L, CH_G], [1, L]]),
                            scalar=0.0,
                            in1=unf(px_f[mh], g0, CH_G),
                            op0=ALU.add, op1=ALU.add)
                        att.append(ah)

                    def ln_fold(src_pair, smp_tag):
                        """Stats for LN(src): returns (xr_pair, mr) where
                        xr = src * rstd_bc and mr row 0 = mean*rstd."""
                        a2 = []
                        for hh in range(2):
                            t = p2.tile([128, CH], bf16, tag="rhs", bufs=6,
                                        name=f"a2_{smp_tag}{hh}")
                            nc.scalar.activation(out=t, in_=src_pair[hh], func=AF.Square)
                            a2.append(t)
                        mean_s = p2.tile([128, CH], f32, tag="stat", bufs=5, name="mean_s")
                        msq_s = p2.tile([128, CH], f32, tag="stat", bufs=5, name="msq_s")
                        for dst, srcs in ((mean_s, src_pair), (msq_s, a2)):
                            for su in range(N_SUBW):
                                cs = slice(su * SUBW, (su + 1) * SUBW)
                                pmn = pst.tile([128, SUBW], f32, tag="stat", name="pmn")
                                for hh in range(2):
                                    nc.tensor.matmul(pmn, onesC_b, srcs[hh][:, cs],
                                                     start=(hh == 0), stop=(hh == 1))
                                nc.scalar.activation(out=dst[:, cs], in_=pmn, func=AF.Copy)
                        m2_ = p2.tile([128, CH], f32, tag="stat", bufs=5, name="m2_")
                        nc.scalar.activation(out=m2_, in_=mean_s, func=AF.Square)
                        var_ = p2.tile([128, CH], f32, tag="stat", bufs=5, name="var_")
                        nc.vector.scalar_tensor_tensor(
                            out=var_, in0=msq_s, scalar=1.0, in1=m2_,
                            op0=ALU.mult, op1=ALU.subtract)
                        sd_ = p2.tile([128, CH], f32, tag="stat", bufs=5, name="lnv_")
                        nc.scalar.activation(out=sd_, in_=var_, func=AF.Ln, bias=eps_t)
                        r_ = p2.tile([128, CH], bf16, tag="rr", bufs=4, name="r_")
                        nc.scalar.activation(out=r_, in_=sd_, func=AF.Exp, scale=-0.5)
                        xr = []
                        for hh in range(2):
                            t = p2.tile([128, CH], bf16, tag="rhs", bufs=6,
                                        name=f"xr_{smp_tag}{hh}")
                            nc.vector.scalar_tensor_tensor(
                                out=t, in0=src_pair[hh], scalar=1.0, in1=r_,
                                op0=ALU.mult, op1=ALU.mult)
                            xr.append(t)
                        mr = p2.tile([128, CH], bf16, tag="rr", bufs=4, name="mr")
                        nc.vector.scalar_tensor_tensor(
                            out=mr, in0=mean_s, scalar=1.0, in1=r_,
                            op0=ALU.mult, op1=ALU.mult)
                        return xr, mr

                    # FFN: y = gelu(attr@W1g - mr*G1 + B1) @ W2 + b2 + att
                    attr, mr1 = ln_fold(att, "f")
                    p1_ = layer(attr, w1g, outer_row=(negg[0], mr1))
                    g1 = []
                    for mh in range(2):
                        gh = p2.tile([128, CH], bf16, tag="rhs", bufs=6, name="g1")
                        nc.scalar.activation(out=gh, in_=p1_[mh], func=AF.Gelu,
                                             bias=B1_t[:, mh:mh + 1])
                        g1.append(gh)
                    p2_ = layer(g1, w2b)
                    y = []
                    for mh in range(2):
                        yh = p2.tile([128, CH], bf16, tag="ytag", bufs=2, name="y")
                        nc.vector.scalar_tensor_tensor(
                            out=yh, in0=p2_[mh],
                            scalar=bias["b2"][:, mh:mh + 1],
                            in1=att[mh], op0=ALU.add, op1=ALU.add)
                        y.append(yh)

                    # temporal: h = gelu(yr@c1g - mr*Gc1 + Bc1), w-major layout
                    yr, mr2 = ln_fold(y, "t")
                    p3_ = layer(yr, c1g, outer_row=(negg[1], mr2))
                    h_act = []
                    for mh in range(2):
                        hh_ = p2.tile([128, CH], f32, tag="hact", bufs=2, name="h_act")
                        dst = _view(hh_, 0, [[V, CH_G], [CH_G * V, W], [1, V]])
                        nc.scalar.activation(out=_r(dst), in_=p3_[mh], func=AF.Gelu,
                                             bias=Bc1_t[:, mh:mh + 1])
                        h_act.append(hh_)

                    # c2: contract (w, i) -> out [O, CH_G*V]
                    for mh in range(2):
                        po = pst.tile([128, CH_G * V], f32, tag="stat", name="po")
                        first = True
                        for w in range(W):
                            for kh in range(2):
                                rhs = h_act[kh][:, w * CH_G * V:(w + 1) * CH_G * V]
                                nc.tensor.matmul(po, _r(c2t[w][kh][:, mh * 128:(mh + 1) * 128]),
                                                 _r(rhs), start=first,
                                                 stop=(w == W - 1 and kh == 1))
                                first = False
                        os_ = p2.tile([128, CH_G * V], f32, tag="os", bufs=2, name="os_")
                        nc.scalar.activation(out=os_, in_=po, func=AF.Identity,
                                             bias=bias["c2_b"][:, mh:mh + 1])
                        nc.sync.dma_start(
                            out=out_d[mh * 128:(mh + 1) * 128, g0:g0 + CH_G, :],
                            in_=os_)

                # software pipeline: emit chunk cc+1's PE-light front before
                # chunk cc's PE-heavy back so every engine's in-order stream
                # interleaves independent work
                zs = {c: chunk_front(c) for c in range(2)}
                for cc in range(N_CH):
                    if cc + 2 < N_CH:
                        zs[cc + 2] = chunk_front(cc + 2)
                    chunk_back(cc, zs.pop(cc))
    return nc


_CACHE = {}


def _get_compiled():
    if "nc" not in _CACHE:
        nc = bacc.Bacc("TRN2", target_bir_lowering=False, debug=False)
        build(nc)
        nc.compile()
        _CACHE["nc"] = nc
    return _CACHE["nc"]


def kernel(**inputs):
    nc = _get_compiled()
    x = np.asarray(inputs["x"], dtype=np.float32)
    n = x.shape[0]
    names = ["Wq", "Wk", "Wv", "Wt", "Wp", "W1", "W2", "c1_w", "Wqa", "Wka",
             "c2_w", "ln1_g", "ln1_b", "bq", "bk", "bv", "bt", "bp", "ffn_g",
             "ffn_b", "b1", "b2", "tn_g", "tn_b", "c1_b", "c2_b", "bqa", "bka"]
    shared = {nm: np.asarray(inputs[nm], dtype=np.float32) for nm in names}
    in_maps = [{"x": x[i], **shared} for i in range(n)]
    res = bass_utils.run_bass_kernel_spmd(nc, in_maps, core_ids=list(range(n)))
    return np.stack([res.results[i]["out"] for i in range(n)], axis=0)


if __name__ == "__main__":
    nc = bacc.Bacc("TRN2", target_bir_lowering=False, debug=False)
    build(nc)
    nc.compile()
    print("build+compile OK")



# revision 8
# speedup vs baseline: 1.1038x; 1.1038x over previous
"""Trainium2 Bass kernel for nn_Block_47193100648803.

Contract: kernel(**inputs) takes FULL unsharded inputs (numpy), returns the
FULL (N, O, T, V) output. Internally shards data-parallel over N across the
8 NeuronCores (one batch element per core, weights replicated).

Layout: channels on SBUF partitions (C=256 -> 2 half-tiles of 128), tokens on
the free axis. The temporal-window unfold is expressed with overlapping
strided access patterns (no data movement). LayerNorm stats are computed with
PE ones-matmuls (sum + partition-broadcast fused); the FFN/temporal LNs are
folded into the following matmul's weights (W1g = g*W1 plus an outer-product
mean correction accumulated in PSUM). Softmaxes run in group-on-partition
layout reached via DRAM-bounce transpose DMAs. Wt@Wp is pre-fused. All
matmuls run fp32r (full PE rate); bf16 only for non-matmul operand storage.
"""

import os
import sys

import numpy as np

for _p in ("/opt/trn_rl_repo", "/root/.axon_site/_ro/trn_rl_repo"):
    if os.path.isdir(_p) and _p not in sys.path:
        sys.path.append(_p)

import concourse.bass as bass
import concourse.tile as tile
from concourse import bacc, bass_utils, mybir
from concourse.masks import make_identity

f32 = mybir.dt.float32
f32r = mybir.dt.float32r
bf16 = mybir.dt.bfloat16
AF = mybir.ActivationFunctionType
ALU = mybir.AluOpType
AX = mybir.AxisListType

# ---- problem constants (hardcoded per spec) ----
N_CORES = 8
C, T, V = 256, 128, 25
H = 8
W = 3
O = 256
L = W * V                 # 75
FT = T + 2                # 130 padded frames
F = FT * V                # 3250 real frame columns (zero pads at both ends)
F_PAD = 3328              # allocated frame columns (8 * 416, fp32r-even subs)
G = T                     # 128 groups per core
GL = G * L                # 9600 group-stage columns
SCALE = 1.0 / (32.0 ** 0.5)
EPS = 1e-5

FSUB = 416                # phase-1 matmul column tile (even, 256..512)
N_FSUB = F_PAD // FSUB    # 8
CH_G = 16                 # groups per chunk in phase 2
N_CH = G // CH_G          # 8
CH = CH_G * L             # 1200
SUB_G = 4                 # groups per matmul sub-tile
SUB = SUB_G * L           # 300
N_SUB = CH_G // SUB_G     # 4
SUBW = 400                # wide matmul sub (fp32r-even, 256..512)
N_SUBW = CH // SUBW       # 3
# bank-aligned dst slices for chunk-wide [128, CH] psum accumulation
BANK_SUBS = [(0, 512), (512, 512), (1024, 176)]


def _r(ap):
    return ap.bitcast(f32r)


def _view(t, offset, dims):
    """AP view on tile t: partition dim kept, free dims replaced."""
    return bass.AP(tensor=t.tensor, offset=t.offset + offset, ap=[t.ap[0]] + dims)


def unf(t, g0, gc):
    """Overlapping window view [128, gc, W, V] on a [128, F] frame tile."""
    return _view(t, g0 * V, [[V, gc], [V, W], [1, V]])


def seg(t, g0, gc):
    """[128, gc, L] view on a [128, GL] or chunk tile starting at group g0
    (g0 relative to tile origin)."""
    return _view(t, g0 * L, [[L, gc], [1, L]])


def bc_g(t, g0, gc):
    """Broadcast per-(c,g) [128, G] tile over L -> [128, gc, L] (step-0)."""
    return _view(t, g0, [[1, gc], [0, L]])


def build(nc):
    x_d = nc.dram_tensor("x", [C, T, V], f32, kind="ExternalInput").ap()
    wd = {}
    for nm in ["Wq", "Wk", "Wv", "Wt", "Wp", "W1", "W2", "c1_w"]:
        wd[nm] = nc.dram_tensor(nm, [C, C], f32, kind="ExternalInput").ap()
    wd["Wqa"] = nc.dram_tensor("Wqa", [C, H], f32, kind="ExternalInput").ap()
    wd["Wka"] = nc.dram_tensor("Wka", [C, H], f32, kind="ExternalInput").ap()
    wd["c2_w"] = nc.dram_tensor("c2_w", [W, C, O], f32, kind="ExternalInput").ap()
    bnames = ["ln1_g", "ln1_b", "bq", "bk", "bv", "bt", "bp", "ffn_g", "ffn_b",
              "b1", "b2", "tn_g", "tn_b", "c1_b", "c2_b"]
    for nm in bnames:
        wd[nm] = nc.dram_tensor(nm, [C], f32, kind="ExternalInput").ap()
    wd["bqa"] = nc.dram_tensor("bqa", [H], f32, kind="ExternalInput").ap()
    wd["bka"] = nc.dram_tensor("bka", [H], f32, kind="ExternalInput").ap()
    out_d = nc.dram_tensor("out", [O, T, V], f32, kind="ExternalOutput").ap()

    qa_d = nc.dram_tensor("qa_scr", [H, F_PAD], f32).ap()
    qw_d = nc.dram_tensor("qw_scr", [H, GL], bf16).ap()
    ka_d = nc.dram_tensor("ka_scr", [H, GL], f32).ap()
    kw_d = nc.dram_tensor("kw_scr", [H, GL], bf16).ap()
    row_d = nc.dram_tensor("row_scr", [3, C], f32).ap()

    with tile.TileContext(nc) as tc:
        with (
            tc.tile_pool(name="consts", bufs=1) as cp,
            tc.tile_pool(name="data", bufs=1) as dp,
        ):
            # ---------- input load first (weights go on other DMA queues) ----
            p1x_cm = tc.tile_pool(name="p1_x", bufs=1)
            p1x = p1x_cm.__enter__()
            x_f = [p1x.tile([128, F_PAD], f32, tag=f"x_f{hh}", name=f"x_f{hh}")
                   for hh in range(2)]
            qa_f = p1x.tile([H, F_PAD], f32, tag="qa_f", name="qa_f")
            for hh in range(2):
                nc.gpsimd.dma_start(out=_r(x_f[hh][:, V:F - V]),
                                    in_=_r(x_d[hh * 128:(hh + 1) * 128, :, :]))

            # ---------- weights / constants ----------
            # f32 masters live in a staging pool freed after setup; only the
            # bf16 working copies persist.
            wsp_cm = tc.tile_pool(name="wstage", bufs=1)
            wsp = wsp_cm.__enter__()
            wt = {}
            for nm in ["Wq", "Wk", "Wv", "Wp", "W1", "W2", "c1_w"]:
                wt[nm] = [wsp.tile([128, C], f32, tag=f"w_{nm}{kh}", name=f"w_{nm}{kh}")
                          for kh in range(2)]
                for kh in range(2):
                    nc.scalar.dma_start(out=_r(wt[nm][kh]),
                                        in_=_r(wd[nm][kh * 128:(kh + 1) * 128, :]))
            for nm in ["Wqa", "Wka"]:
                wt[nm] = [wsp.tile([128, H], f32, tag=f"w_{nm}{kh}", name=f"w_{nm}{kh}")
                          for kh in range(2)]
                for kh in range(2):
                    nc.scalar.dma_start(out=_r(wt[nm][kh]),
                                        in_=_r(wd[nm][kh * 128:(kh + 1) * 128, :]))
            c2t = []
            for w in range(W):
                c2t.append([wsp.tile([128, O], f32, tag=f"w_c2_{w}{kh}", name=f"w_c2_{w}{kh}")
                            for kh in range(2)])
                for kh in range(2):
                    nc.scalar.dma_start(out=_r(c2t[w][kh]),
                                        in_=_r(wd["c2_w"][w, kh * 128:(kh + 1) * 128, :]))

            def load_bias_col(nm):
                t = cp.tile([128, 2], f32, tag=f"b_{nm}", name=f"b_{nm}")
                src = bass.AP(tensor=wd[nm].tensor, offset=wd[nm].offset,
                              ap=[[1, 128], [128, 2]])
                nc.scalar.dma_start(out=t, in_=src)
                return t

            bias = {nm: load_bias_col(nm) for nm in bnames}
            for nm in ["bqa", "bka"]:
                t = cp.tile([H, 1], f32, tag=f"b_{nm}", name=f"b_{nm}")
                nc.sync.dma_start(out=t, in_=wd[nm])
                bias[nm] = t

            eps_t = cp.tile([128, 1], f32, tag="eps", name="eps_t")
            nc.vector.memset(eps_t, EPS)

            def fill_r(t, value):
                # constant fill with an f32r-typed output (plain Memset cannot
                # emit f32r): Copy(in*0 + value) ignores the uninitialized in_
                nc.scalar.activation(out=_r(t), in_=_r(t), func=AF.Copy,
                                     bias=float(value), scale=0.0)

            onesC = cp.tile([128, 128], f32, tag="onesC", name="onesC")
            fill_r(onesC, 1.0 / C)
            onesC_b = cp.tile([128, 128], bf16, tag="onesC_b", name="onesC_b")
            nc.scalar.activation(out=onesC_b, in_=onesC, func=AF.Copy)
            w2b = [cp.tile([128, C], bf16, tag=f"w2b{kh}", name=f"w2b{kh}") for kh in range(2)]
            wkab = [cp.tile([128, H], bf16, tag=f"wkab{kh}", name=f"wkab{kh}") for kh in range(2)]
            negones = cp.tile([128, 1], f32, tag="negones", name="negones")
            fill_r(negones, -1.0)
            negones_b = cp.tile([128, 1], bf16, tag="negones_b", name="negones_b")
            nc.scalar.activation(out=negones_b, in_=negones, func=AF.Copy)

            # folded weights: W1g = ffn_g*W1, c1g = tn_g*c1_w
            w1g = [cp.tile([128, C], bf16, tag=f"w1g{kh}", name=f"w1g{kh}") for kh in range(2)]
            c1g = [cp.tile([128, C], bf16, tag=f"c1g{kh}", name=f"c1g{kh}") for kh in range(2)]
            for kh in range(2):
                nc.vector.tensor_scalar_mul(w1g[kh], wt["W1"][kh], bias["ffn_g"][:, kh:kh + 1])
                nc.vector.tensor_scalar_mul(c1g[kh], wt["c1_w"][kh], bias["tn_g"][:, kh:kh + 1])
                nc.scalar.activation(out=w2b[kh], in_=wt["W2"][kh], func=AF.Copy)
                nc.scalar.activation(out=wkab[kh], in_=wt["Wka"][kh], func=AF.Copy)

            # bf16 copies of the phase-1 matmul weights (full-rate PE)
            wb = {}
            for nm in ["Wq", "Wk", "Wv", "Wp"]:
                wb[nm] = [cp.tile([128, C], bf16, tag=f"wb_{nm}{kh}", name=f"wb_{nm}{kh}")
                          for kh in range(2)]
                for kh in range(2):
                    nc.scalar.activation(out=wb[nm][kh], in_=wt[nm][kh], func=AF.Copy)
            wqab = [cp.tile([128, H], bf16, tag=f"wqab{kh}", name=f"wqab{kh}") for kh in range(2)]
            for kh in range(2):
                nc.scalar.activation(out=wqab[kh], in_=wt["Wqa"][kh], func=AF.Copy)
            c2b = []
            for w in range(W):
                c2b.append([cp.tile([128, O], bf16, tag=f"wb_c2_{w}{kh}", name=f"wb_c2_{w}{kh}")
                            for kh in range(2)])
                for kh in range(2):
                    nc.scalar.activation(out=c2b[w][kh], in_=c2t[w][kh], func=AF.Copy)

            wtp = [cp.tile([128, C], bf16, tag=f"wtp{kh}", name=f"wtp{kh}") for kh in range(2)]
            negg = [cp.tile([1, C], bf16, tag=f"negg{i}", name=f"negg{i}")
                    for i in range(2)]  # [-G1], [-Gc1]

            # ---------- setup-scoped: Wtp = Wt@Wp, bias rows ----------
            with (
                tc.tile_pool(name="setup_sb", bufs=1) as sp,
                tc.tile_pool(name="setup_ps", bufs=2, space="PSUM") as spp,
            ):
                wtw = [sp.tile([128, C], f32, tag=f"wt{kh}", name=f"wtw{kh}")
                       for kh in range(2)]
                for kh in range(2):
                    nc.sync.dma_start(out=wtw[kh],
                                      in_=wd["Wt"][kh * 128:(kh + 1) * 128, :])
                ident = sp.tile([128, 128], f32, tag="ident", name="ident")
                make_identity(nc, ident)

                for kh in range(2):
                    pacc = spp.tile([128, C], f32, tag="wtp_acc", name="pacc")
                    for mh in range(2):
                        ptr = spp.tile([128, 128], f32, tag="tr", name="ptr")
                        nc.tensor.transpose(ptr, wtw[kh][:, mh * 128:(mh + 1) * 128], ident)
                        a_t = sp.tile([128, 128], f32, tag="a_t", name="a_t")
                        nc.scalar.activation(out=_r(a_t), in_=ptr, func=AF.Copy)
                        nc.tensor.matmul(pacc, _r(a_t), _r(wt["Wp"][mh]),
                                         start=(mh == 0), stop=(mh == 1))
                    nc.scalar.activation(out=wtp[kh], in_=pacc, func=AF.Copy)

                def colvec(nm, kh):
                    t = sp.tile([128, 1], f32, tag=f"cv_{nm}{kh}", name=f"cv_{nm}{kh}")
                    src = bass.AP(tensor=wd[nm].tensor, offset=wd[nm].offset + kh * 128,
                                  ap=[[1, 128], [128, 1]])
                    nc.sync.dma_start(out=_r(t), in_=_r(src))
                    return t

                def rowvec(nm):
                    t = sp.tile([1, C], f32, tag=f"rv_{nm}", name=f"rv_{nm}")
                    nc.sync.dma_start(out=t, in_=wd[nm])
                    return t

                for i, (bnm, wmat, addnm) in enumerate([
                    ("bt", wt["Wp"], "bp"),
                    ("ffn_b", wt["W1"], "b1"),
                    ("tn_b", wt["c1_w"], "c1_b"),
                ]):
                    pr = spp.tile([1, C], f32, tag="rowacc", name="pr")
                    for kh in range(2):
                        nc.tensor.matmul(pr, _r(colvec(bnm, kh)), _r(wmat[kh]),
                                         start=(kh == 0), stop=(kh == 1))
                    row_i = sp.tile([1, C], f32, tag=f"row_i{i}", name=f"row_i{i}")
                    nc.vector.tensor_add(row_i, pr, rowvec(addnm))
                    nc.sync.dma_start(out=row_d[i:i + 1, :], in_=row_i)

                for i, wmat in enumerate([w1g, c1g]):
                    pg = spp.tile([1, C], f32, tag="rowacc", name="pg")
                    for kh in range(2):
                        nc.tensor.matmul(pg, negones_b, wmat[kh],
                                         start=(kh == 0), stop=(kh == 1))
                    nc.scalar.activation(out=negg[i], in_=pg, func=AF.Copy)

            wsp_cm.__exit__(None, None, None)

            # bounce bias rows back into per-partition [128, 2] layout
            btp_t = cp.tile([128, 2], f32, tag="btp", name="btp_t")
            B1_t = cp.tile([128, 2], f32, tag="B1", name="B1_t")
            Bc1_t = cp.tile([128, 2], f32, tag="Bc1", name="Bc1_t")
            for i, t in enumerate([btp_t, B1_t, Bc1_t]):
                src = bass.AP(tensor=row_d.tensor, offset=row_d.offset + i * C,
                              ap=[[1, 128], [128, 1]])
                nc.sync.dma_start(out=t[:, 0:1], in_=src)
                src2 = bass.AP(tensor=row_d.tensor, offset=row_d.offset + i * C + 128,
                               ap=[[1, 128], [128, 1]])
                nc.sync.dma_start(out=t[:, 1:2], in_=src2)

            # ---------- persistent activations ----------
            q_f = [dp.tile([128, F_PAD], bf16, tag=f"q_f{hh}", name=f"q_f{hh}") for hh in range(2)]
            k_f = [dp.tile([128, F_PAD], bf16, tag=f"k_f{hh}", name=f"k_f{hh}") for hh in range(2)]
            v_f = [dp.tile([128, F_PAD], bf16, tag=f"v_f{hh}", name=f"v_f{hh}") for hh in range(2)]
            px_f = [dp.tile([128, F_PAD], bf16, tag=f"px_f{hh}", name=f"px_f{hh}") for hh in range(2)]
            pq_t = [dp.tile([128, G], f32, tag=f"pq{hh}", name=f"pq{hh}") for hh in range(2)]
            pk_t = [dp.tile([128, G], f32, tag=f"pk{hh}", name=f"pk{hh}") for hh in range(2)]
            pq_b = [dp.tile([128, G], bf16, tag=f"pqb{hh}", name=f"pqb{hh}") for hh in range(2)]
            pk_b = [dp.tile([128, G], bf16, tag=f"pkb{hh}", name=f"pkb{hh}") for hh in range(2)]

            # ---------- phase 1: per-frame pipeline ----------
            with (
                tc.tile_pool(name="p1_sb", bufs=2) as p1,
                tc.tile_pool(name="p1_ps", bufs=1, space="PSUM") as pp1,
                tc.tile_pool(name="p1_mm", bufs=4, space="PSUM") as pp1m,
            ):
                for hh in range(2):
                    fill_r(x_f[hh][:, 0:V], 0.0)
                    fill_r(x_f[hh][:, F - V:F_PAD], 0.0)

                for s in range(N_FSUB):
                    sl = slice(s * FSUB, (s + 1) * FSUB)
                    x2 = [p1.tile([128, FSUB], f32, tag=f"x2_{hh}", name=f"x2_{hh}")
                          for hh in range(2)]
                    for hh in range(2):
                        nc.vector.scalar_tensor_tensor(
                            out=_r(x2[hh]), in0=x_f[hh][:, sl], scalar=1.0,
                            in1=x_f[hh][:, sl], op0=ALU.mult, op1=ALU.mult)
                    pmean = pp1.tile([128, FSUB], f32, tag="pmean", name="pmean")
                    pmsq = pp1.tile([128, FSUB], f32, tag="pmsq", name="pmsq")
                    for hh in range(2):
                        nc.tensor.matmul(pmean, _r(onesC), _r(x_f[hh][:, sl]),
                                         start=(hh == 0), stop=(hh == 1))
                    for hh in range(2):
                        nc.tensor.matmul(pmsq, _r(onesC), _r(x2[hh]),
                                         start=(hh == 0), stop=(hh == 1))
                    m2 = p1.tile([128, FSUB], f32, tag="m2", name="m2")
                    nc.scalar.activation(out=m2, in_=pmean, func=AF.Square)
                    var = p1.tile([128, FSUB], f32, tag="var", name="var")
                    nc.vector.tensor_sub(var, pmsq, m2)
                    srt = p1.tile([128, FSUB], f32, tag="sd", name="srt")
                    nc.scalar.activation(out=srt, in_=var, func=AF.Sqrt, bias=eps_t)
                    rstd = p1.tile([128, FSUB], f32, tag="rstd", name="rstd")
                    nc.vector.reciprocal_approx_fast(out=rstd, in_=srt)
                    nx = []
                    for hh in range(2):
                        xc = p1.tile([128, FSUB], f32, tag=f"xc{hh}", name=f"xc{hh}")
                        nc.vector.tensor_sub(xc, x_f[hh][:, sl], pmean)
                        xg = p1.tile([128, FSUB], f32, tag=f"xg{hh}", name=f"xg{hh}")
                        nc.vector.scalar_tensor_tensor(
                            out=xg, in0=xc, scalar=bias["ln1_g"][:, hh:hh + 1],
                            in1=rstd, op0=ALU.mult, op1=ALU.mult)
                        nxh = p1.tile([128, FSUB], bf16, tag=f"nx{hh}", name=f"nx{hh}")
                        nc.vector.tensor_scalar_add(nxh, xg, bias["ln1_b"][:, hh:hh + 1])
                        nx.append(nxh)
                    for mh in range(2):
                        pq_ = pp1m.tile([128, FSUB], f32, tag="mm", name="pq_")
                        for kh in range(2):
                            nc.tensor.matmul(pq_, wb["Wq"][kh][:, mh * 128:(mh + 1) * 128],
                                             nx[kh], start=(kh == 0), stop=(kh == 1))
                        nc.scalar.activation(out=q_f[mh][:, sl], in_=pq_, func=AF.Identity,
                                             bias=bias["bq"][:, mh:mh + 1])
                    for nm, bnm, dst in [("Wk", "bk", k_f), ("Wv", "bv", v_f)]:
                        for mh in range(2):
                            pm_ = pp1m.tile([128, FSUB], f32, tag="mm", name="pm_")
                            for kh in range(2):
                                nc.tensor.matmul(pm_,
                                                 wb[nm][kh][:, mh * 128:(mh + 1) * 128],
                                                 nx[kh], start=(kh == 0), stop=(kh == 1))
                            nc.scalar.activation(out=dst[mh][:, sl], in_=pm_,
                                                 func=AF.Identity,
                                                 bias=bias[bnm][:, mh:mh + 1])
                    pqa = pp1.tile([H, FSUB], f32, tag="pqa", name="pqa")
                    for kh in range(2):
                        nc.tensor.matmul(pqa, wqab[kh], nx[kh],
                                         start=(kh == 0), stop=(kh == 1))
                    nc.scalar.activation(out=qa_f[:, sl], in_=pqa, func=AF.Identity,
                                         bias=bias["bqa"])
                    # px = q@Wp + btp + x   (pre-added residual path for attn)
                    for mh in range(2):
                        pp_ = pp1m.tile([128, FSUB], f32, tag="mm", name="pp_")
                        for kh in range(2):
                            nc.tensor.matmul(pp_, wb["Wp"][kh][:, mh * 128:(mh + 1) * 128],
                                             q_f[kh][:, sl], start=(kh == 0), stop=(kh == 1))
                        nc.vector.scalar_tensor_tensor(
                            out=px_f[mh][:, sl], in0=pp_, scalar=btp_t[:, mh:mh + 1],
                            in1=x_f[mh][:, sl], op0=ALU.add, op1=ALU.add)
                nc.sync.dma_start(out=qa_d, in_=qa_f)

            p1x_cm.__exit__(None, None, None)

            # ---------- global qw softmax (batched over all groups) ----------
            with tc.tile_pool(name="smq", bufs=1) as smq:
                ag = smq.tile([G, H * L], f32, tag="ag", name="ag_q")
                qa_gather_all = bass.AP(
                    tensor=qa_d.tensor, offset=qa_d.offset,
                    ap=[[V, G], [F_PAD, H], [V, W], [1, V]])
                nc.gpsimd.dma_start(out=ag, in_=qa_gather_all)
                ag3 = _view(ag, 0, [[L, H], [1, L]])
                mx = smq.tile([G, H], f32, tag="mx", name="mx_q")
                nc.vector.reduce_max(mx, ag3, axis=AX.X)
                e = smq.tile([G, H * L], f32, tag="e", name="e_q")
                nc.vector.tensor_sub(_view(e, 0, [[L, H], [1, L]]), ag3,
                                     _view(mx, 0, [[1, H], [0, L]]))
                nc.scalar.activation(out=e, in_=e, func=AF.Exp, scale=SCALE)
                sm = smq.tile([G, H], f32, tag="sm", name="sm_q")
                nc.vector.reduce_sum(sm, _view(e, 0, [[L, H], [1, L]]), axis=AX.X)
                rs = smq.tile([G, H], f32, tag="rs", name="rs_q")
                nc.vector.reciprocal(rs, sm)
                wgn = smq.tile([G, H * L], bf16, tag="wgn", name="wgn_q")
                nc.vector.scalar_tensor_tensor(
                    out=_view(wgn, 0, [[L, H], [1, L]]),
                    in0=_view(e, 0, [[L, H], [1, L]]), scalar=1.0,
                    in1=_view(rs, 0, [[1, H], [0, L]]),
                    op0=ALU.mult, op1=ALU.mult)
                qw_all = bass.AP(tensor=qw_d.tensor, offset=qw_d.offset,
                                 ap=[[L, G], [GL, H], [1, L]])
                nc.gpsimd.dma_start(out=qw_all, in_=wgn)

            # ---------- phase 2: unified per-chunk pipeline ----------
            with (
                tc.tile_pool(name="p2_sb", bufs=1) as p2,
                tc.tile_pool(name="p2_ps", bufs=2, space="PSUM") as pmm,
                tc.tile_pool(name="p2_ps2", bufs=2, space="PSUM") as pst,
            ):
                def softmax_chunk(src_gather_ap, dst_dram, g0, tagp):
                    """Per-chunk softmax in [128 = 16 groups x 8 heads, L]
                    layout; writes normalized weights to dst_dram[h, cols]."""
                    ag = p2.tile([128, L], f32, tag="sm_ag", bufs=6,
                                 name=f"ag_{tagp}")
                    nc.gpsimd.dma_start(out=ag, in_=src_gather_ap)
                    mx = p2.tile([128, 1], f32, tag="sm_mx", bufs=6,
                                 name=f"mx_{tagp}")
                    nc.vector.reduce_max(mx, ag, axis=AX.X)
                    e = p2.tile([128, L], f32, tag="sm_e", bufs=6,
                                name=f"e_{tagp}")
                    nc.vector.tensor_scalar_sub(e, ag, mx[:, 0:1])
                    nc.scalar.activation(out=e, in_=e, func=AF.Exp, scale=SCALE)
                    sm = p2.tile([128, 1], f32, tag="sm_s", bufs=6,
                                 name=f"sm_{tagp}")
                    nc.vector.reduce_sum(sm, e, axis=AX.X)
                    rs = p2.tile([128, 1], f32, tag="sm_rs", bufs=6,
                                 name=f"rs_{tagp}")
                    nc.vector.reciprocal(rs, sm)
                    wgn = p2.tile([128, L], bf16, tag="sm_w", bufs=6,
                                  name=f"wgn_{tagp}")
                    nc.vector.tensor_scalar_mul(wgn, e, rs[:, 0:1])
                    dst = bass.AP(tensor=dst_dram.tensor,
                                  offset=dst_dram.offset + g0 * L,
                                  ap=[[L, CH_G], [GL, H], [1, L]])
                    nc.gpsimd.dma_start(out=dst, in_=wgn)

                def head_bcast(src_dram, g0, hh, tagp):
                    """[128, CH] tile with partition c reading
                    src_dram[c // 32 (+4*hh), chunk cols] via broadcast DMA."""
                    t = p2.tile([128, CH], bf16, tag="bc", bufs=6,
                                name=f"bc_{tagp}")
                    src = bass.AP(
                        tensor=src_dram.tensor,
                        offset=src_dram.offset + (hh * 4) * GL + g0 * L,
                        ap=[[GL, 4], [0, 32], [1, CH]])
                    nc.sync.dma_start(out=t, in_=src)
                    return t

                def chunk_front(cc):
                    g0 = cc * CH_G
                    col0 = g0 * L

                    # pooled query pq, then kp = k * pq, ka = kp @ Wka
                    kp = []
                    for hh in range(2):
                        qb = head_bcast(qw_d, g0, hh, f"q{hh}")
                        prod = p2.tile([128, CH], bf16, tag="prod", bufs=4,
                                       name="prod")
                        nc.vector.scalar_tensor_tensor(
                            out=_view(prod, 0, [[L, CH_G], [1, L]]),
                            in0=unf(q_f[hh], g0, CH_G), scalar=1.0,
                            in1=_view(qb, 0, [[L, CH_G], [1, L]]),
                            op0=ALU.mult, op1=ALU.mult)
                        nc.vector.reduce_sum(pq_t[hh][:, g0:g0 + CH_G],
                                             _view(prod, 0, [[L, CH_G], [1, L]]),
                                             axis=AX.X)
                        nc.vector.tensor_copy(pq_b[hh][:, g0:g0 + CH_G],
                                              pq_t[hh][:, g0:g0 + CH_G])
                        kph = p2.tile([128, CH], bf16, tag="rhs", bufs=6, name="kph")
                        nc.vector.scalar_tensor_tensor(
                            out=_view(kph, 0, [[L, CH_G], [1, L]]),
                            in0=unf(k_f[hh], g0, CH_G), scalar=1.0,
                            in1=bc_g(pq_b[hh], g0, CH_G),
                            op0=ALU.mult, op1=ALU.mult)
                        kp.append(kph)
                    ka_c = p2.tile([H, CH], f32, tag="ka_c", bufs=2, name="ka_c")
                    for su in range(N_SUBW):
                        pka = pst.tile([H, SUBW], f32, tag="stat", name="pka")
                        for kh in range(2):
                            nc.tensor.matmul(pka, wkab[kh],
                                             kp[kh][:, su * SUBW:(su + 1) * SUBW],
                                             start=(kh == 0), stop=(kh == 1))
                        nc.scalar.activation(out=ka_c[:, su * SUBW:(su + 1) * SUBW],
                                             in_=pka, func=AF.Identity, bias=bias["bka"])
                    nc.gpsimd.dma_start(out=ka_d[:, col0:col0 + CH], in_=ka_c)

                    # kw softmax for this chunk
                    ka_gather = bass.AP(
                        tensor=ka_d.tensor, offset=ka_d.offset + col0,
                        ap=[[L, CH_G], [GL, H], [1, L]])
                    softmax_chunk(ka_gather, kw_d, g0, "k")

                    # pooled key pk, z = v * pk
                    z = []
                    for hh in range(2):
                        kb = head_bcast(kw_d, g0, hh, f"k{hh}")
                        prod = p2.tile([128, CH], bf16, tag="prod", bufs=4,
                                       name="prod2")
                        nc.vector.scalar_tensor_tensor(
                            out=_view(prod, 0, [[L, CH_G], [1, L]]),
                            in0=unf(k_f[hh], g0, CH_G), scalar=1.0,
                            in1=_view(kb, 0, [[L, CH_G], [1, L]]),
                            op0=ALU.mult, op1=ALU.mult)
                        nc.vector.reduce_sum(pk_t[hh][:, g0:g0 + CH_G],
                                             _view(prod, 0, [[L, CH_G], [1, L]]),
                                             axis=AX.X)
                        nc.vector.tensor_copy(pk_b[hh][:, g0:g0 + CH_G],
                                              pk_t[hh][:, g0:g0 + CH_G])
                        zh = p2.tile([128, CH], bf16, tag="ztag", bufs=6, name="zh")
                        nc.vector.scalar_tensor_tensor(
                            out=_view(zh, 0, [[L, CH_G], [1, L]]),
                            in0=unf(v_f[hh], g0, CH_G), scalar=1.0,
                            in1=bc_g(pk_b[hh], g0, CH_G),
                            op0=ALU.mult, op1=ALU.mult)
                        z.append(zh)
                    return z

                def chunk_back(cc, z):
                    g0 = cc * CH_G
                    col0 = g0 * L

                    def layer(rhs_pair, wpair, outer_row=None):
                        """Chunk-wide psum per out-half of rhs @ W (+ optional
                        K=1 outer-product accumulation); bank-aligned dst
                        slices, kh-outer for stationary reuse."""
                        ps = []
                        for mh in range(2):
                            pm = pmm.tile([128, CH], f32, tag="mm", bufs=2, name="pm")
                            last = outer_row is None
                            for kh in range(2):
                                for o0, w_ in BANK_SUBS:
                                    cs = slice(o0, o0 + w_)
                                    nc.tensor.matmul(
                                        pm[:, cs],
                                        wpair[kh][:, mh * 128:(mh + 1) * 128],
                                        rhs_pair[kh][:, cs],
                                        start=(kh == 0), stop=(kh == 1) and last)
                            if outer_row is not None:
                                row, vec = outer_row
                                for o0, w_ in BANK_SUBS:
                                    cs = slice(o0, o0 + w_)
                                    nc.tensor.matmul(
                                        pm[:, cs],
                                        row[0:1, mh * 128:(mh + 1) * 128],
                                        vec[0:1, cs],
                                        start=False, stop=True)
                            ps.append(pm)
                        return ps

                    # att = z @ Wtp + px_unf
                    patt = layer(z, wtp)
                    att = []
                    for mh in range(2):
                        ah = p2.tile([128, CH], bf16, tag="att", bufs=2, name="att")
                        nc.vector.scalar_tensor_tensor(
                            out=_view(ah, 0, [[L, CH_G], [1, L]]),
                            in0=_view(patt[mh], 0, [[L, CH_G], [1, L]]),
                            scalar=0.0,
                            in1=unf(px_f[mh], g0, CH_G),
                            op0=ALU.add, op1=ALU.add)
                        att.append(ah)

                    def ln_stats(src_pair, smp_tag):
                        """LN stats for src (pair of [128,CH] bf16): returns
                        (rstd [128,CH] f32 bc over partitions, mean_sb [1,CH]
                        bf16). Normalization is applied AFTER the following
                        matmul: (src@Wg - mean*colsum(Wg)) * rstd."""
                        a2 = []
                        for hh in range(2):
                            t = p2.tile([128, CH], bf16, tag="rhs", bufs=6,
                                        name=f"a2_{smp_tag}{hh}")
                            nc.vector.tensor_mul(t, src_pair[hh], src_pair[hh])
                            a2.append(t)
                        var_ = p2.tile([128, CH], f32, tag="stat", bufs=5,
                                       name=f"var_{smp_tag}")
                        mean_sb = p2.tile([1, CH], bf16, tag="meanrow", bufs=4,
                                          name=f"mean_{smp_tag}")
                        for su in range(N_SUBW):
                            cs = slice(su * SUBW, (su + 1) * SUBW)
                            pmn = pst.tile([128, SUBW], f32, tag="stat", name="pmn")
                            for hh in range(2):
                                nc.tensor.matmul(pmn, onesC_b, src_pair[hh][:, cs],
                                                 start=(hh == 0), stop=(hh == 1))
                            pms = pst.tile([128, SUBW], f32, tag="stat", name="pms")
                            for hh in range(2):
                                nc.tensor.matmul(pms, onesC_b, a2[hh][:, cs],
                                                 start=(hh == 0), stop=(hh == 1))
                            nc.vector.tensor_copy(mean_sb[0:1, cs], pmn[0:1, :])
                            m2s = p2.tile([128, SUBW], f32, tag="m2s", bufs=4,
                                          name=f"m2s_{smp_tag}")
                            nc.scalar.activation(out=m2s, in_=pmn, func=AF.Square)
                            nc.vector.tensor_sub(var_[:, cs], pms, m2s)
                        srt = p2.tile([128, CH], f32, tag="stat", bufs=5,
                                      name=f"srt_{smp_tag}")
                        nc.scalar.activation(out=srt, in_=var_, func=AF.Sqrt,
                                             bias=eps_t)
                        rstd = p2.tile([128, CH], f32, tag="stat", bufs=5,
                                       name=f"rstd_{smp_tag}")
                        nc.vector.reciprocal_approx_fast(out=rstd, in_=srt)
                        return rstd, mean_sb

                    # FFN: y = gelu((att@W1g - mean*G1)*rstd + B1) @ W2 + b2 + att
                    rstd1, mean1 = ln_stats(att, "f")
                    p1_ = layer(att, w1g, outer_row=(negg[0], mean1))
                    g1 = []
                    for mh in range(2):
                        tg = p2.tile([128, CH], bf16, tag="rhs", bufs=6, name="tg")
                        nc.vector.scalar_tensor_tensor(
                            out=tg, in0=p1_[mh], scalar=1.0, in1=rstd1,
                            op0=ALU.mult, op1=ALU.mult)
                        gh = p2.tile([128, CH], bf16, tag="rhs", bufs=6, name="g1")
                        nc.scalar.activation(out=gh, in_=tg, func=AF.Gelu,
                                             bias=B1_t[:, mh:mh + 1])
                        g1.append(gh)
                    p2_ = layer(g1, w2b)
                    y = []
                    for mh in range(2):
                        yh = p2.tile([128, CH], bf16, tag="ytag", bufs=2, name="y")
                        nc.vector.scalar_tensor_tensor(
                            out=yh, in0=p2_[mh],
                            scalar=bias["b2"][:, mh:mh + 1],
                            in1=att[mh], op0=ALU.add, op1=ALU.add)
                        y.append(yh)

                    # temporal: h = gelu((y@c1g - mean*Gc1)*rstd + Bc1), w-major
                    rstd2, mean2 = ln_stats(y, "t")
                    p3_ = layer(y, c1g, outer_row=(negg[1], mean2))
                    h_act = []
                    for mh in range(2):
                        tg2 = p2.tile([128, CH], bf16, tag="rhs", bufs=6, name="tg2")
                        nc.vector.scalar_tensor_tensor(
                            out=tg2, in0=p3_[mh], scalar=1.0, in1=rstd2,
                            op0=ALU.mult, op1=ALU.mult)
                        hh_ = p2.tile([128, CH], bf16, tag="hact", bufs=2, name="h_act")
                        dst = _view(hh_, 0, [[V, CH_G], [CH_G * V, W], [1, V]])
                        nc.scalar.activation(out=dst, in_=tg2, func=AF.Gelu,
                                             bias=Bc1_t[:, mh:mh + 1])
                        h_act.append(hh_)

                    # c2: contract (w, i) -> out [O, CH_G*V]
                    for mh in range(2):
                        po = pst.tile([128, CH_G * V], f32, tag="stat", name="po")
                        first = True
                        for w in range(W):
                            for kh in range(2):
                                rhs = h_act[kh][:, w * CH_G * V:(w + 1) * CH_G * V]
                                nc.tensor.matmul(po, c2b[w][kh][:, mh * 128:(mh + 1) * 128],
                                                 rhs, start=first,
                                                 stop=(w == W - 1 and kh == 1))
                                first = False
                        os_ = p2.tile([128, CH_G * V], f32, tag="os", bufs=2, name="os_")
                        nc.scalar.activation(out=os_, in_=po, func=AF.Identity,
                                             bias=bias["c2_b"][:, mh:mh + 1])
                        nc.sync.dma_start(
                            out=out_d[mh * 128:(mh + 1) * 128, g0:g0 + CH_G, :],
                            in_=os_)

                # software pipeline: emit chunk cc+1's PE-light front before
                # chunk cc's PE-heavy back so every engine's in-order stream
                # interleaves independent work
                zs = {c: chunk_front(c) for c in range(2)}
                for cc in range(N_CH):
                    if cc + 2 < N_CH:
                        zs[cc + 2] = chunk_front(cc + 2)
                    chunk_back(cc, zs.pop(cc))
    return nc


_CACHE = {}


def _get_compiled():
    if "nc" not in _CACHE:
        nc = bacc.Bacc("TRN2", target_bir_lowering=False, debug=False)
        build(nc)
        nc.compile()
        _CACHE["nc"] = nc
    return _CACHE["nc"]


def kernel(**inputs):
    nc = _get_compiled()
    x = np.asarray(inputs["x"], dtype=np.float32)
    n = x.shape[0]
    names = ["Wq", "Wk", "Wv", "Wt", "Wp", "W1", "W2", "c1_w", "Wqa", "Wka",
             "c2_w", "ln1_g", "ln1_b", "bq", "bk", "bv", "bt", "bp", "ffn_g",
             "ffn_b", "b1", "b2", "tn_g", "tn_b", "c1_b", "c2_b", "bqa", "bka"]
    shared = {nm: np.asarray(inputs[nm], dtype=np.float32) for nm in names}
    in_maps = [{"x": x[i], **shared} for i in range(n)]
    res = bass_utils.run_bass_kernel_spmd(nc, in_maps, core_ids=list(range(n)))
    return np.stack([res.results[i]["out"] for i in range(n)], axis=0)


if __name__ == "__main__":
    nc = bacc.Bacc("TRN2", target_bir_lowering=False, debug=False)
    build(nc)
    nc.compile()
    print("build+compile OK")

